# revision 1
# baseline (speedup 1.0000x reference)
"""Conv3D (stride (1,2,2), pad (2,3,3)) as a Bass/Tile kernel for 8 trn2 cores.

Problem: x (8,3,16,112,112) f32, weight (64,3,5,7,7), bias (64,)
      -> out (8,64,16,56,56).  Data-parallel: one batch sample per core.

Device strategy (per core, per output depth od):
  The contraction over (c=3, kw=7, kd=5) = 105 terms is packed on the PE
  partition axis (plus a constant-ones row carrying the bias), and the
  remaining kernel dim kh=7 is a PSUM accumulation loop.  The stride-2
  output-row walk rides a strided access pattern; the kw (stride-2 cols)
  and kd (depth) shifts cannot vary per-partition in one AP, so the host
  pre-builds a duplicated layout

     R[od, p=(c,kw,kd), hp, j] = Xpad[c, od+kd, hp, 2*j+kw]   (+ ones row)

  and the kernel streams matmuls
     psum[o, oh, j] (+)= sum_p W[p, kh, o] * R[od, p, 2*oh+kh, j]
  as fp32r (fp22 multiplies, fp32 accumulate, full PE rate at N=448).

  DMAs are issued on nc.gpsimd (SWDGE) — one dma_start fans out across all
  16 SDMA engines, ~340 GB/s vs ~52 GB/s for a single HWDGE queue.
"""

import numpy as np

import concourse.bass as bass
import concourse.mybir as mybir
import concourse.tile as tile
from concourse import bacc
from concourse.bass_utils import run_bass_kernel_spmd

N, C, D, H, W = 8, 3, 16, 112, 112
O, KD, KH, KW = 64, 5, 7, 7
PD, PH, PW = 2, 3, 3
OD, OH, OW = 16, 56, 56
KP = C * KW * KD          # 105 contraction rows
NP = KP + 1               # + ones row for bias
HP = H + 2 * PH           # 118 padded input rows
OHB = 8                   # output rows per psum half-tile
OHC = OH // OHB           # 7 chunks per od: 3 column-paired + 1 solo

_CACHE = {}
LAST_RUN = None


def _build_bass():
    nc = bacc.Bacc("TRN2", target_bir_lowering=False, debug=False, num_devices=N)
    f32 = mybir.dt.float32
    f32r = mybir.dt.float32r
    r = nc.dram_tensor("r", [OD, NP, HP, OW], f32r, kind="ExternalInput")
    w = nc.dram_tensor("w", [NP, KH, O], f32r, kind="ExternalInput")
    out = nc.dram_tensor("out", [O, OD, OHC, OHB, OW], f32, kind="ExternalOutput")

    with tile.TileContext(nc) as tc:
        with (
            tc.tile_pool(name="wp", bufs=1) as wp,
            tc.tile_pool(name="sp", bufs=4) as sp,
            tc.tile_pool(name="op", bufs=3) as op,
            tc.tile_pool(name="pp", bufs=8, space=bass.MemorySpace.PSUM) as pp,
        ):
            wt = wp.tile([NP, KH, O], f32r)
            nc.gpsimd.dma_start(wt[:], w[:])
            for od in range(OD):
                s = sp.tile([NP, HP, OW], f32r)
                nc.gpsimd.dma_start(s[:], r[od])
                ob = op.tile([O, OHC, OHB, OW], f32)

                def rhs(t, kh):
                    base = 2 * OHB * t + kh
                    return s[0:NP, base : base + 2 * OHB : 2, :]

                for t in range(OHC):
                    ps = pp.tile([O, OHB, OW], f32)
                    for kh in range(KH):
                        nc.tensor.matmul(
                            ps[:], wt[0:NP, kh, :], rhs(t, kh),
                            start=(kh == 0), stop=(kh == KH - 1),
                        )
                    if t % 2 == 0:
                        nc.scalar.copy(ob[0:O, t], ps[:])
                    else:
                        nc.vector.tensor_copy(ob[0:O, t], ps[:])
                nc.gpsimd.dma_start(out[0:O, od], ob[:])
    nc.compile()
    return nc


def _host_pack(x, weight, bias):
    """Build the pre-shifted rhs volume R per sample and the weight tiles."""
    xf = np.ascontiguousarray(x, dtype=np.float32)
    xp = np.zeros((N, C, D + 2 * PD, HP, W + 2 * PW), np.float32)
    xp[:, :, PD : PD + D, PH : PH + H, PW : PW + W] = xf

    R = np.empty((N, OD, NP, HP, OW), np.float32)
    p = 0
    for c in range(C):
        for kw in range(KW):
            for kd in range(KD):
                # R[n, od, p, hp, j] = xp[n, c, od+kd, hp, 2*j+kw]
                R[:, :, p] = xp[:, c, kd : kd + OD, :, kw : kw + 2 * OW : 2]
                p += 1
    R[:, :, KP] = 1.0

    # Wt[p=(c,kw,kd), kh, o]; ones row carries bias on kh=0 only.
    Wt = np.zeros((NP, KH, O), np.float32)
    Wt[:KP] = (
        np.asarray(weight, np.float32)
        .transpose(1, 4, 2, 3, 0)  # [C, KW, KD, KH, O]
        .reshape(KP, KH, O)
    )
    Wt[KP, 0] = np.asarray(bias, np.float32)
    return R, Wt


def kernel(x, weight, bias):
    global LAST_RUN
    if "nc" not in _CACHE:
        _CACHE["nc"] = _build_bass()
    nc = _CACHE["nc"]

    R, Wt = _host_pack(x, weight, bias)
    in_maps = [{"r": R[n], "w": Wt} for n in range(N)]
    res = run_bass_kernel_spmd(nc, in_maps, core_ids=list(range(N)))
    LAST_RUN = res
    out = np.stack([res.results[n]["out"].reshape(O, OD, OH, OW) for n in range(N)], axis=0)
    return out.astype(np.float32, copy=False)



# revision 2
# speedup vs baseline: 2.0707x; 2.0707x over previous
"""Conv3D (stride (1,2,2), pad (2,3,3)) as a Bass/Tile kernel for 8 trn2 cores.

Problem: x (8,3,16,112,112) f32, weight (64,3,5,7,7), bias (64,)
      -> out (8,64,16,56,56).  Data-parallel: one batch sample per core.

Per core the contraction over (c=3, kw=7, kd=5) = 105 terms (+ a ones row
carrying the bias) rides the PE partition axis; kh=7 is a PSUM accumulation
loop.  The host pre-builds the kw/kd-shifted, stride-2-selected volume

   R[od, p=(c,kw,kd), hp, j] = Xpad[c, od+kd, hp, 2*j+kw]   (+ ones row)

in bf16 (fp32r matmuls stream at ~2 cycles/row on real HW; bf16 streams at
1 cycle/row and halves all DMA traffic; rel-err ~0.5% << the 2e-2 gate).

The 56 output rows per od split into 8 tiles of 7 rows; tile pairs are
issued as concurrent 128x64 column-tiled matmuls (tile_position (0,0) and
(0,64)) so the 64-channel output only idles half the PE array, not all of
it.  PSUM [128,392] f32 accumulates over kh, is cast-copied to bf16 in
SBUF (scalar/vector alternating), and written out via the HWDGE queue
(nc.sync) while input R chunks (2 od per dma_start, 2.8 MB) stream in on
the SWDGE queue (nc.gpsimd) - separate queues so loads and stores don't
serialize behind each other in one FIFO.
"""

import numpy as np
import ml_dtypes

import concourse.bass as bass
import concourse.mybir as mybir
import concourse.tile as tile
from concourse import bacc
from concourse.bass_utils import run_bass_kernel_spmd

N, C, D, H, W = 8, 3, 16, 112, 112
O, KD, KH, KW = 64, 5, 7, 7
PD, PH, PW = 2, 3, 3
OD, OH, OW = 16, 56, 56
KP = C * KW * KD          # 105 contraction rows
NP = KP + 1               # + ones row for bias
HP = H + 2 * PH           # 118 padded input rows
OHB = 7                   # output rows per matmul tile
T = OH // OHB             # 8 tiles -> 4 column-tile pairs per od
PAIRS = T // 2
NF = OHB * OW             # 392 moving free size per matmul
ODC = OD // 2             # 8 two-od input chunks

BF16 = mybir.dt.bfloat16

_CACHE = {}
LAST_RUN = None


def _build_bass():
    nc = bacc.Bacc("TRN2", target_bir_lowering=False, debug=False, num_devices=N)
    f32 = mybir.dt.float32
    r = nc.dram_tensor("r", [ODC, NP, 2, HP, OW], BF16, kind="ExternalInput")
    w = nc.dram_tensor("w", [NP, KH, O], BF16, kind="ExternalInput")
    out = nc.dram_tensor("out", [128, ODC, 2, PAIRS, NF], BF16, kind="ExternalOutput")

    with tile.TileContext(nc) as tc:
        with (
            tc.tile_pool(name="wp", bufs=1) as wp,
            tc.tile_pool(name="sp", bufs=3) as sp,
            tc.tile_pool(name="op", bufs=3) as op,
            tc.tile_pool(name="pp", bufs=8, space=bass.MemorySpace.PSUM) as pp,
        ):
            wt = wp.tile([NP, KH, O], BF16)
            nc.scalar.dma_start(wt[:], w[:])
            cp = 0
            for j in range(ODC):
                s = sp.tile([NP, 2, HP, OW], BF16)
                nc.gpsimd.dma_start(s[:], r[j])
                ob = op.tile([128, 2, PAIRS, NF], BF16)
                for i in range(2):
                    for k in range(PAIRS):
                        ps = pp.tile([128, NF], f32)
                        bA = 28 * k          # tile t=2k   -> input rows 14t+kh
                        bB = 28 * k + 14     # tile t=2k+1
                        for kh in range(KH):
                            nc.tensor.matmul(
                                ps[0:O, :], wt[0:NP, kh, :],
                                s[0:NP, i, bA + kh : bA + kh + 14 : 2, :],
                                start=(kh == 0), stop=(kh == KH - 1),
                            )
                            nc.tensor.matmul(
                                ps[O:128, :], wt[0:NP, kh, :],
                                s[0:NP, i, bB + kh : bB + kh + 14 : 2, :],
                                start=(kh == 0), stop=(kh == KH - 1),
                            )
                        if cp % 2 == 0:
                            nc.scalar.copy(ob[0:128, i, k], ps[:])
                        else:
                            nc.vector.tensor_copy(ob[0:128, i, k], ps[:])
                        cp += 1
                nc.sync.dma_start(out[0:128, j], ob[:])
    nc.compile()
    return nc


def _host_pack(x, weight, bias):
    """Pre-shifted rhs volume R per sample (bf16) and the weight tiles."""
    xf = np.ascontiguousarray(x, dtype=np.float32)
    xp = np.zeros((N, C, D + 2 * PD, HP, W + 2 * PW), np.float32)
    xp[:, :, PD : PD + D, PH : PH + H, PW : PW + W] = xf

    R = np.empty((N, ODC, NP, 2, HP, OW), np.float32)
    p = 0
    for c in range(C):
        for kw in range(KW):
            for kd in range(KD):
                for i in range(2):
                    # od = 2j+i ; depth = od+kd ; R[n,j,p,i] = xp[n,c,2j+i+kd,:,kw::2]
                    R[:, :, p, i] = (
                        xp[:, c, i + kd : i + kd + OD : 2, :, kw : kw + 2 * OW : 2]
                    )
                p += 1
    R[:, :, KP] = 1.0
    Rb = R.astype(ml_dtypes.bfloat16)

    Wt = np.zeros((NP, KH, O), np.float32)
    Wt[:KP] = (
        np.asarray(weight, np.float32)
        .transpose(1, 4, 2, 3, 0)  # [C, KW, KD, KH, O]
        .reshape(KP, KH, O)
    )
    Wt[KP, 0] = np.asarray(bias, np.float32)
    return Rb, Wt.astype(ml_dtypes.bfloat16)


def kernel(x, weight, bias):
    global LAST_RUN
    if "nc" not in _CACHE:
        _CACHE["nc"] = _build_bass()
    nc = _CACHE["nc"]

    Rb, Wt = _host_pack(x, weight, bias)
    in_maps = [{"r": Rb[n], "w": Wt} for n in range(N)]
    res = run_bass_kernel_spmd(nc, in_maps, core_ids=list(range(N)))
    LAST_RUN = res
    outs = []
    for n in range(N):
        arr = np.asarray(res.results[n]["out"]).astype(np.float32)
        # [128, ODC, 2, PAIRS, NF] -> [half,o, j,i, k, r,w] -> (O, OD, OH, OW)
        arr = arr.reshape(2, O, ODC, 2, PAIRS, OHB, OW)
        arr = arr.transpose(1, 2, 3, 4, 0, 5, 6).reshape(O, OD, OH, OW)
        outs.append(arr)
    return np.stack(outs, axis=0)
